# revision 1
# baseline (speedup 1.0000x reference)
"""GAT (4-layer, shared-weight) Trainium2 kernel over 8 NeuronCores.

Sharding: destination-node sharding. Core c owns nodes [c*6250, (c+1)*6250)
(padded to 6272) and all edges whose dst lands in that range (edges sorted by
dst on the host). Per layer:
  - every core computes the full h' = X @ [W_att | V_src | V_dst] table
    (replicated compute; V_* fold the attention vectors so per-node attention
    scalars come out of the same matmul) and writes it to its DRAM,
  - each core gathers h'[src] for its edges (indirect DMA), builds one-hot
    dst-selection matrices on-chip, and computes the segment softmax +
    weighted aggregation with TensorE matmuls accumulating in PSUM.
    Softmax max-subtraction is skipped (logits are in [-0.6, 2.4] for this
    model; exp is safe) and the normalization division is applied once per
    destination row after aggregation.
  - residual update for owned rows, then AllGather of X^T for the next layer.
"""
import os
import sys

sys.path.insert(0, "/opt/trn_rl_repo")

import numpy as np

N, NFEAT, NHID, NCLASS, NLAYERS, NHEADS = 50000, 256, 64, 40, 4, 4
NEG_SLOPE = 0.2
NCORES = 8
NPER = N // NCORES           # 6250 real nodes per core
NPAD = 6272                  # padded (49 * 128)
NTILES = NPAD // 128         # 49 dst tiles per core
NTAB = NPAD * NCORES         # 50176 padded global nodes
NTG = NTAB // 128            # 392 global node tiles
HW = NHEADS * NHID + 2 * NHEADS  # 264 = h(256) | a_src(4) | a_dst(4)
PADDST = -1000.0


def _pad_gid(n):
    """real node id -> padded global row id"""
    return (n // NPER) * NPAD + (n % NPER)


def _prepare(inputs):
    x = np.asarray(inputs["x"], np.float32)
    ei = np.asarray(inputs["edge_index"], np.int64)
    W_enc = np.asarray(inputs["W_enc"], np.float32)
    b_enc = np.asarray(inputs["b_enc"], np.float32)
    W_att = np.asarray(inputs["W_att"], np.float32)
    att_src = np.asarray(inputs["att_src"], np.float32)
    att_dst = np.asarray(inputs["att_dst"], np.float32)
    b_att = np.asarray(inputs["b_att"], np.float32)
    W_dec = np.asarray(inputs["W_dec"], np.float32)
    b_dec = np.asarray(inputs["b_dec"], np.float32)

    # fold attention vectors: a_src = X @ Vsrc with Vsrc[64,4]
    Vsrc = np.stack(
        [W_att[:, h * NHID:(h + 1) * NHID] @ att_src[h] for h in range(NHEADS)], 1
    ).astype(np.float32)
    Vdst = np.stack(
        [W_att[:, h * NHID:(h + 1) * NHID] @ att_dst[h] for h in range(NHEADS)], 1
    ).astype(np.float32)
    Wbig = np.concatenate([W_att, Vsrc, Vdst], axis=1)  # [64, 264]

    # edges + self loops, sorted by dst
    src = np.concatenate([ei[0], np.arange(N, dtype=np.int64)])
    dst = np.concatenate([ei[1], np.arange(N, dtype=np.int64)])
    order = np.argsort(dst, kind="stable")
    src, dst = src[order], dst[order]

    # per (core, tile) edge counts -> shared subtile counts S[t]
    core_of = dst // NPER
    loc = dst - core_of * NPER          # local dst id 0..6249
    tile_of = loc // 128
    intile = (loc - tile_of * 128).astype(np.float32)

    counts = np.zeros((NCORES, NTILES), np.int64)
    np.add.at(counts, (core_of, tile_of), 1)
    S = np.maximum(np.ceil(counts.max(axis=0) / 128).astype(np.int64), 1)  # [NTILES]
    NSUB = int(S.sum())

    esrc = np.zeros((NCORES, NSUB * 128), np.int32)       # padded-gid gather rows
    edst = np.full((NCORES, NSUB * 128), PADDST, np.float32)  # in-tile dst or -1000
    sub_base = np.concatenate([[0], np.cumsum(S)[:-1]])   # subtile base per tile
    for c in range(NCORES):
        m = core_of == c
        sc, tc_, ic = src[m], tile_of[m], intile[m]
        ord2 = np.argsort(tc_, kind="stable")
        sc, tc_, ic = sc[ord2], tc_[ord2], ic[ord2]
        bounds = np.searchsorted(tc_, np.arange(NTILES + 1))
        for t in range(NTILES):
            lo, hi = bounds[t], bounds[t + 1]
            base = sub_base[t] * 128
            esrc[c, base:base + hi - lo] = _pad_gid(sc[lo:hi])
            edst[c, base:base + hi - lo] = ic[lo:hi]
    # [128, NSUB] layout: column k holds subtile k's 128 edges on partitions
    esrc = esrc.reshape(NCORES, NSUB, 128).transpose(0, 2, 1).copy()
    edst = edst.reshape(NCORES, NSUB, 128).transpose(0, 2, 1).copy()

    # per-core x^T slices [256, NPAD]
    xT = np.zeros((NCORES, NFEAT, NPAD), np.float32)
    for c in range(NCORES):
        xT[c, :, :NPER] = x[c * NPER:(c + 1) * NPER].T

    iota = np.tile(np.arange(128, dtype=np.float32)[None, :], (128, 1))
    consts = dict(
        wenc=np.concatenate([W_enc[:128], W_enc[128:]], axis=1),  # [128, 128]
        benc=np.tile(b_enc[None, :], (128, 1)),
        wbig=np.concatenate([Wbig, Wbig], axis=0),  # replicated on both halves
        vdst=Vdst,
        batt=np.tile(b_att[None, :], (128, 1)),
        wdec=W_dec,
        bdec=np.tile(b_dec[None, :], (128, 1)),
        iota=iota,
    )
    in_maps = []
    for c in range(NCORES):
        m = dict(consts)
        m["xT"] = xT[c]
        m["esrc"] = esrc[c]
        m["edst"] = edst[c]
        in_maps.append(m)
    return in_maps, [int(v) for v in S], NSUB


def _build(S, NSUB):
    import os
    DBG = int(os.environ.get('GAT_DEBUG', '0'))
    NL = int(os.environ.get('GAT_NL', str(NLAYERS)))
    from concourse import bass, bacc, mybir, tile
    from concourse.masks import make_identity

    f32 = mybir.dt.float32
    i32 = mybir.dt.int32
    AF = mybir.ActivationFunctionType
    OP = mybir.AluOpType
    AX = mybir.AxisListType

    nc = bacc.Bacc("TRN2", target_bir_lowering=False, debug=False,
                   num_devices=NCORES)
    t_xT = nc.declare_dram_parameter("xT", [NFEAT, NPAD], f32, isOutput=False)
    t_esrc = nc.declare_dram_parameter("esrc", [128, NSUB], i32, isOutput=False)
    t_edst = nc.declare_dram_parameter("edst", [128, NSUB], f32, isOutput=False)
    t_wenc = nc.declare_dram_parameter("wenc", [128, 128], f32, isOutput=False)
    t_benc = nc.declare_dram_parameter("benc", [128, NHID], f32, isOutput=False)
    t_wbig = nc.declare_dram_parameter("wbig", [128, HW], f32, isOutput=False)
    t_vdst = nc.declare_dram_parameter("vdst", [NHID, NHEADS], f32, isOutput=False)
    t_batt = nc.declare_dram_parameter("batt", [128, 256], f32, isOutput=False)
    t_wdec = nc.declare_dram_parameter("wdec", [NHID, NCLASS], f32, isOutput=False)
    t_bdec = nc.declare_dram_parameter("bdec", [128, NCLASS], f32, isOutput=False)
    t_iota = nc.declare_dram_parameter("iota", [128, 128], f32, isOutput=False)
    t_out = nc.declare_dram_parameter("out", [NPAD, NCLASS], f32, isOutput=True)

    if int(os.environ.get("GAT_H2OUT", "0")):
        h2 = nc.declare_dram_parameter("h2dump", [NTAB, HW], f32, isOutput=True)
    else:
        h2 = nc.dram_tensor("h2tab", [NTAB, HW], f32)
    ag_in = nc.dram_tensor("ag_in", [NHID, NPAD], f32)
    ag_out = nc.dram_tensor("ag_out", [NCORES * NHID, NPAD], f32,
                            addr_space="Shared")
    sub_base = np.concatenate([[0], np.cumsum(S)[:-1]]).astype(int)

    with tile.TileContext(nc) as tc:
        with tc.tile_pool(name="const", bufs=1) as cp, \
             tc.tile_pool(name="work", bufs=4) as wp, \
             tc.tile_pool(name="gath", bufs=16) as gp, \
             tc.tile_pool(name="pacc", bufs=2, space="PSUM") as pacc, \
             tc.tile_pool(name="prot", bufs=2, space="PSUM") as prot:

            # ---- resident tiles / constants ----
            XT = cp.tile([128, 4 * NPAD], f32)      # X^T, all cores' nodes
            if DBG >= 2 and DBG != 6:
                nc.gpsimd.memset(XT[:], 0.0)
            XTo = cp.tile([NHID, NPAD], f32)        # own X^T
            adst = cp.tile([128, NTILES * NHEADS], f32)
            esrc_sb = cp.tile([128, NSUB], i32)
            edst_sb = cp.tile([128, NSUB], f32)
            wenc_sb = cp.tile([128, 128], f32)
            benc_sb = cp.tile([128, NHID], f32)
            wbig_sb = cp.tile([128, HW], f32)
            vdst_sb = cp.tile([NHID, NHEADS], f32)
            batt_sb = cp.tile([128, 256], f32)
            wdec_sb = cp.tile([NHID, NCLASS], f32)
            bdec_sb = cp.tile([128, NCLASS], f32)
            iota_sb = cp.tile([128, 128], f32)
            ident = cp.tile([128, 128], f32)
            zero4 = cp.tile([128, NHEADS], f32)

            if DBG not in (5, 6):
                nc.sync.dma_start(out=esrc_sb[:], in_=t_esrc[:, :])
                nc.sync.dma_start(out=edst_sb[:], in_=t_edst[:, :])
            nc.sync.dma_start(out=wenc_sb[:], in_=t_wenc[:, :])
            nc.sync.dma_start(out=benc_sb[:], in_=t_benc[:, :])
            nc.sync.dma_start(out=wbig_sb[:], in_=t_wbig[:, :])
            nc.sync.dma_start(out=vdst_sb[:], in_=t_vdst[:, :])
            nc.sync.dma_start(out=batt_sb[:], in_=t_batt[:, :])
            nc.sync.dma_start(out=wdec_sb[:], in_=t_wdec[:, :])
            nc.sync.dma_start(out=bdec_sb[:], in_=t_bdec[:, :])
            nc.sync.dma_start(out=iota_sb[:], in_=t_iota[:, :])
            make_identity(nc, ident[:])
            if DBG != 6:
                nc.gpsimd.memset(zero4[:], 0.0)

            def ts(i, n=128):
                return slice(i * n, (i + 1) * n)

            # ---- encoder: X0 = relu(x @ W_enc + b_enc) for own nodes ----
            for t in range(NTILES):
                pe = pacc.tile([128, HW], f32, tag="pp")
                for k in range(2):
                    lt = wp.tile([128, 128], f32, tag="xload")
                    nc.sync.dma_start(out=lt[:], in_=t_xT[ts(k), ts(t)])
                    nc.tensor.matmul(pe[:, 0:NHID], lhsT=lt[:],
                                     rhs=wenc_sb[:, k * NHID:(k + 1) * NHID],
                                     start=(k == 0), stop=(k == 1))
                x0 = wp.tile([128, NHID], f32, tag="x0")
                nc.vector.tensor_add(out=x0[:], in0=pe[:, 0:NHID], in1=benc_sb[:])
                nc.scalar.activation(out=x0[:], in_=x0[:], func=AF.Relu)
                pt = prot.tile([128, HW], f32, tag="pr")
                nc.tensor.transpose(out=pt[0:NHID, 0:128], in_=x0[:], identity=ident[:])
                nc.vector.tensor_copy(out=XTo[:, ts(t)], in_=pt[0:NHID, 0:128])
            if DBG not in (5, 6):
                nc.sync.dma_start(out=ag_in[:, :], in_=XTo[:])
            if DBG < 2:
                nc.gpsimd.collective_compute(
                    "AllGather", OP.bypass,
                    replica_groups=[list(range(NCORES))],
                    ins=[ag_in.ap().opt()], outs=[ag_out.ap().opt()])
            for r in range(NCORES if DBG < 2 else 0):
                nc.sync.dma_start(
                    out=XT[(r % 2) * NHID:(r % 2 + 1) * NHID,
                           (r // 2) * NPAD:(r // 2 + 1) * NPAD],
                    in_=ag_out[r * NHID:(r + 1) * NHID, :])

            # ---- layers ----
            for L in range(NL if DBG not in (4, 5, 6) else 0):
                # a_dst for own tiles
                for t in range(NTILES):
                    pa = prot.tile([128, HW], f32, tag="pr")
                    nc.tensor.matmul(pa[:, 0:NHEADS], lhsT=XTo[:, ts(t)],
                                     rhs=vdst_sb[:], start=True, stop=True)
                    nc.vector.tensor_copy(out=adst[:, t * NHEADS:(t + 1) * NHEADS],
                                          in_=pa[:, 0:NHEADS])
                # h' table sweep (all 392 global tiles)
                for m in range(NTG):
                    r, lm = m // NTILES, m % NTILES
                    lhsT = XT[:][(r % 2) * NHID:(r % 2 + 1) * NHID,
                                 (r // 2) * NPAD + lm * 128:
                                 (r // 2) * NPAD + (lm + 1) * 128]
                    ph = pacc.tile([128, HW], f32, tag="pp")
                    nc.tensor.matmul(
                        ph[:], lhsT=lhsT,
                        rhs=wbig_sb[:][(r % 2) * NHID:(r % 2 + 1) * NHID, :],
                        start=True, stop=True)
                    hsb = wp.tile([128, HW], f32, tag="hsb")
                    nc.vector.tensor_copy(out=hsb[:], in_=ph[:])
                    nc.sync.dma_start(out=h2[ts(m), :], in_=hsb[:])

                # edge sweep
                for t in range(NTILES if DBG in (0, 3) else 0):
                    po = pacc.tile([128, HW], f32, tag="po")
                    St = S[t]
                    for j in range(St):
                        kk = int(sub_base[t]) + j
                        gt = gp.tile([128, HW], f32, tag="gt")
                        nc.gpsimd.indirect_dma_start(
                            out=gt[:], out_offset=None, in_=h2[:, :],
                            in_offset=bass.IndirectOffsetOnAxis(
                                ap=esrc_sb[:, kk:kk + 1], axis=0))
                        oh = wp.tile([128, 128], f32, tag="oh")
                        nc.vector.tensor_tensor(
                            out=oh[:],
                            in0=edst_sb[:, kk:kk + 1].to_broadcast([128, 128]),
                            in1=iota_sb[:], op=OP.is_equal)
                        ptr = prot.tile([128, HW], f32, tag="pr")
                        nc.tensor.transpose(out=ptr[:, 0:128], in_=oh[:],
                                            identity=ident[:])
                        oht = wp.tile([128, 128], f32, tag="oht")
                        nc.vector.tensor_copy(out=oht[:], in_=ptr[:, 0:128])
                        pad_ = prot.tile([128, HW], f32, tag="pr2")
                        nc.tensor.matmul(pad_[:, 0:NHEADS], lhsT=oht[:],
                                         rhs=adst[:, t * NHEADS:(t + 1) * NHEADS],
                                         start=True, stop=True)
                        wv = wp.tile([128, NHEADS], f32, tag="wv")
                        nc.vector.tensor_add(out=wv[:], in0=gt[:, 256:260],
                                             in1=pad_[:, 0:NHEADS])
                        # ACT Lrelu ignores alpha (fixed 0.01 table); do it exactly
                        wv2 = wp.tile([128, NHEADS], f32, tag="wv2")
                        nc.scalar.activation(out=wv2[:], in_=wv[:], func=AF.Copy,
                                             scale=NEG_SLOPE)
                        nc.vector.tensor_tensor(out=wv[:], in0=wv[:], in1=wv2[:],
                                                op=OP.max)
                        nc.scalar.activation(out=wv[:], in_=wv[:], func=AF.Exp)
                        rhs = wp.tile([128, 260], f32, tag="rhs")
                        nc.vector.tensor_copy(out=rhs[:, 0:NHEADS], in_=wv[:])
                        for hd in range(NHEADS):
                            nc.vector.tensor_mul(
                                out=rhs[:, 4 + hd * NHID:4 + (hd + 1) * NHID],
                                in0=gt[:, hd * NHID:(hd + 1) * NHID],
                                in1=wv[:, hd:hd + 1].to_broadcast([128, NHID]))
                        nc.tensor.matmul(po[:, 0:260], lhsT=oh[:], rhs=rhs[:],
                                         start=(j == 0), stop=(j == St - 1))
                    # epilogue: alpha-normalize, bias, relu, head-mean, update
                    msk = wp.tile([128, NHEADS], f32, tag="msk")
                    nc.vector.tensor_tensor(out=msk[:], in0=po[:, 0:NHEADS],
                                            in1=zero4[:], op=OP.is_equal)
                    nc.vector.tensor_add(out=msk[:], in0=po[:, 0:NHEADS], in1=msk[:])
                    rcp = wp.tile([128, NHEADS], f32, tag="rcp")
                    nc.vector.reciprocal(out=rcp[:], in_=msk[:])
                    conv = wp.tile([128, 256], f32, tag="conv")
                    for hd in range(NHEADS):
                        nc.vector.tensor_mul(
                            out=conv[:, hd * NHID:(hd + 1) * NHID],
                            in0=po[:, 4 + hd * NHID:4 + (hd + 1) * NHID],
                            in1=rcp[:, hd:hd + 1].to_broadcast([128, NHID]))
                    nc.vector.tensor_add(out=conv[:], in0=conv[:], in1=batt_sb[:])
                    nc.scalar.activation(out=conv[:], in_=conv[:], func=AF.Relu)
                    upd = wp.tile([128, NHID], f32, tag="upd")
                    nc.vector.tensor_reduce(
                        out=upd[:], in_=conv[:].rearrange("p (j u) -> p j u", u=4),
                        axis=AX.X, op=OP.add)
                    nc.scalar.activation(out=upd[:], in_=upd[:], func=AF.Copy,
                                         scale=0.25)
                    ptu = prot.tile([128, HW], f32, tag="pr")
                    nc.tensor.transpose(out=ptu[0:NHID, 0:128], in_=upd[:],
                                        identity=ident[:])
                    nc.vector.tensor_add(out=XTo[:, ts(t)], in0=XTo[:, ts(t)],
                                         in1=ptu[0:NHID, 0:128])
                if L < NL - 1 and DBG < 2:
                    nc.sync.dma_start(out=ag_in[:, :], in_=XTo[:])
                    nc.gpsimd.collective_compute(
                        "AllGather", OP.bypass,
                        replica_groups=[list(range(NCORES))],
                        ins=[ag_in.ap().opt()], outs=[ag_out.ap().opt()])
                    for r in range(NCORES):
                        nc.sync.dma_start(
                            out=XT[(r % 2) * NHID:(r % 2 + 1) * NHID,
                                   (r // 2) * NPAD:(r // 2 + 1) * NPAD],
                            in_=ag_out[r * NHID:(r + 1) * NHID, :])

            # ---- decoder ----
            for t in range(NTILES):
                pd = pacc.tile([128, HW], f32, tag="pp")
                nc.tensor.matmul(pd[:, 0:NCLASS], lhsT=XTo[:, ts(t)],
                                 rhs=wdec_sb[:], start=True, stop=True)
                dsb = wp.tile([128, NCLASS], f32, tag="dsb")
                nc.vector.tensor_add(out=dsb[:], in0=pd[:, 0:NCLASS], in1=bdec_sb[:])
                nc.sync.dma_start(out=t_out[ts(t), :], in_=dsb[:])

    nc.compile()
    return nc


def kernel(**inputs) -> np.ndarray:
    from concourse.bass_utils import run_bass_kernel_spmd

    in_maps, S, NSUB = _prepare(inputs)
    nc = _build(S, NSUB)
    trace = os.environ.get("GAT_TRACE", "0") == "1"
    ncores_run = int(os.environ.get("GAT_CORES", NCORES))
    r = run_bass_kernel_spmd(nc, in_maps[:ncores_run],
                             core_ids=list(range(ncores_run)), trace=trace)
    if trace and r.exec_time_ns is not None:
        print(f"HW exec time: {r.exec_time_ns} ns")
    if os.environ.get("GAT_H2OUT", "0") == "1":
        np.save("/root/problem/h2dump.npy", r.results[0]["h2dump"])
    out = np.concatenate(
        [r.results[c]["out"][:NPER] for c in range(len(r.results))], axis=0)
    if len(r.results) < NCORES:
        out = np.pad(out, ((0, N - out.shape[0]), (0, 0)))
    return out.astype(np.float32)


if __name__ == "__main__":
    import reference as R
    inputs = R.setup_inputs()
    out = kernel(**{k: np.asarray(v) for k, v in inputs.items()})
    print("out shape:", out.shape)



# revision 7
# speedup vs baseline: 1.2769x; 1.2769x over previous
"""GAT (4-layer, shared-weight) Trainium2 kernel over 8 NeuronCores.

Sharding: destination-node sharding. Core c owns nodes [c*6250, (c+1)*6250)
(padded to 6272) and all edges whose dst lands in that range (edges sorted by
dst on the host). Per layer:
  - every core computes the full h' table (h interleaved with const-1 columns
    per head, plus per-node a_src) in bf16 and writes it to its DRAM,
  - each core gathers h'[src] for its edges (indirect DMA, one op per
    128-edge subtile), builds one-hot dst-selection matrices on-chip, and
    computes the segment softmax + weighted aggregation with bf16 TensorE
    matmuls accumulating in fp32 PSUM. Softmax max-subtraction is skipped
    (logits are bounded for this model); the normalization division is
    applied once per destination row after aggregation. The (h|1)-interleaved
    table layout lets one broadcast-multiply produce both the weighted
    features and the softmax-denominator columns of the aggregation RHS.
  - residual update for owned rows, then AllGather of X^T (bf16) for the
    next layer.
"""
import os
import sys

sys.path.insert(0, "/opt/trn_rl_repo")

import numpy as np

N, NFEAT, NHID, NCLASS, NLAYERS, NHEADS = 50000, 256, 64, 40, 4, 4
NEG_SLOPE = 0.2
NCORES = 8
NPER = N // NCORES           # 6250 real nodes per core
NPAD = 6272                  # padded (49 * 128)
NTILES = NPAD // 128         # 49 dst tiles per core
NTAB = NPAD * NCORES         # 50176 padded global nodes
NTG = NTAB // 128            # 392 global node tiles
HW = NHEADS * (NHID + 1) + NHEADS  # 264 = (h|1)x4 (260) | a_src (4)
PADDST = -1000.0


def _pad_gid(n):
    """real node id -> padded global row id"""
    return (n // NPER) * NPAD + (n % NPER)


def _prepare(inputs):
    import ml_dtypes
    bf = ml_dtypes.bfloat16

    x = np.asarray(inputs["x"], np.float32)
    ei = np.asarray(inputs["edge_index"], np.int64)
    W_enc = np.asarray(inputs["W_enc"], np.float32)
    b_enc = np.asarray(inputs["b_enc"], np.float32)
    W_att = np.asarray(inputs["W_att"], np.float32)
    att_src = np.asarray(inputs["att_src"], np.float32)
    att_dst = np.asarray(inputs["att_dst"], np.float32)
    b_att = np.asarray(inputs["b_att"], np.float32)
    W_dec = np.asarray(inputs["W_dec"], np.float32)
    b_dec = np.asarray(inputs["b_dec"], np.float32)

    # fold attention vectors: a_src = X @ Vsrc with Vsrc[64,4]
    Vsrc = np.stack(
        [W_att[:, h * NHID:(h + 1) * NHID] @ att_src[h] for h in range(NHEADS)], 1
    ).astype(np.float32)
    Vdst = np.stack(
        [W_att[:, h * NHID:(h + 1) * NHID] @ att_dst[h] for h in range(NHEADS)], 1
    ).astype(np.float32)

    # interleaved [(h_h | 0) x 4 | a_src] fold; the 0 column per head is
    # overwritten with a constant 1 after the sweep matmul so the po matmul
    # accumulates softmax denominators alongside the weighted features.
    WbigI = np.zeros((NHID, HW), np.float32)
    for h in range(NHEADS):
        WbigI[:, h * (NHID + 1):h * (NHID + 1) + NHID] = \
            W_att[:, h * NHID:(h + 1) * NHID]
    WbigI[:, NHEADS * (NHID + 1):] = Vsrc
    wbigA = np.concatenate([WbigI, np.zeros((NHID, HW), np.float32)], 0)
    wbigB = np.concatenate([np.zeros((NHID, HW), np.float32), WbigI], 0)

    # edges + self loops, sorted by dst
    src = np.concatenate([ei[0], np.arange(N, dtype=np.int64)])
    dst = np.concatenate([ei[1], np.arange(N, dtype=np.int64)])
    order = np.argsort(dst, kind="stable")
    src, dst = src[order], dst[order]

    # per (core, tile) edge counts -> shared subtile counts S[t]
    core_of = dst // NPER
    loc = dst - core_of * NPER          # local dst id 0..6249
    tile_of = loc // 128
    intile = (loc - tile_of * 128).astype(np.float32)

    counts = np.zeros((NCORES, NTILES), np.int64)
    np.add.at(counts, (core_of, tile_of), 1)
    S = np.maximum(np.ceil(counts.max(axis=0) / 128).astype(np.int64), 1)  # [NTILES]
    NSUB = int(S.sum())

    esrc = np.zeros((NCORES, NSUB * 128), np.int32)       # padded-gid gather rows
    edst = np.full((NCORES, NSUB * 128), PADDST, np.float32)  # in-tile dst or -1000
    sub_base = np.concatenate([[0], np.cumsum(S)[:-1]])   # subtile base per tile
    for c in range(NCORES):
        m = core_of == c
        sc, tc_, ic = src[m], tile_of[m], intile[m]
        ord2 = np.argsort(tc_, kind="stable")
        sc, tc_, ic = sc[ord2], tc_[ord2], ic[ord2]
        bounds = np.searchsorted(tc_, np.arange(NTILES + 1))
        for t in range(NTILES):
            lo, hi = bounds[t], bounds[t + 1]
            base = sub_base[t] * 128
            esrc[c, base:base + hi - lo] = _pad_gid(sc[lo:hi])
            edst[c, base:base + hi - lo] = ic[lo:hi]
    # [128, NSUB] layout: column k holds subtile k's 128 edges on partitions
    esrc = esrc.reshape(NCORES, NSUB, 128).transpose(0, 2, 1).copy()
    edst = edst.reshape(NCORES, NSUB, 128).transpose(0, 2, 1).copy()

    # per-core x^T slices [256, NPAD] (bf16 for the encoder matmul)
    xT = np.zeros((NCORES, NFEAT, NPAD), np.float32)
    for c in range(NCORES):
        xT[c, :, :NPER] = x[c * NPER:(c + 1) * NPER].T

    iota = np.tile(np.arange(128, dtype=np.float32)[None, :], (128, 1))
    consts = dict(
        wenc=np.concatenate([W_enc[:128], W_enc[128:]], axis=1).astype(bf),
        benc=np.tile(b_enc[None, :], (128, 1)).astype(np.float32),
        wbigA=wbigA.astype(bf),
        wbigB=wbigB.astype(bf),
        vdst=Vdst,
        batt=np.tile(b_att[None, :], (128, 1)).astype(np.float32),
        wdec=W_dec,
        bdec=np.tile(b_dec[None, :], (128, 1)).astype(np.float32),
        iota=iota,
        ones4=np.ones((128, NHEADS), np.float32),
    )
    in_maps = []
    for c in range(NCORES):
        m = dict(consts)
        m["xT"] = xT[c].astype(bf)
        m["esrc"] = esrc[c]
        m["edst"] = edst[c]
        in_maps.append(m)
    return in_maps, [int(v) for v in S], NSUB


def _build(S, NSUB):
    DBG = int(os.environ.get('GAT_DEBUG', '0'))
    NL = int(os.environ.get('GAT_NL', str(NLAYERS)))
    from concourse import bass, bacc, mybir, tile
    from concourse.masks import make_identity

    f32 = mybir.dt.float32
    bf16 = mybir.dt.bfloat16
    i32 = mybir.dt.int32
    AF = mybir.ActivationFunctionType
    OP = mybir.AluOpType
    AX = mybir.AxisListType
    HP = NHID + 1  # 65: per-head (h|1) stride in the table row

    nc = bacc.Bacc("TRN2", target_bir_lowering=False, debug=False,
                   num_devices=NCORES)
    t_xT = nc.declare_dram_parameter("xT", [NFEAT, NPAD], bf16, isOutput=False)
    t_esrc = nc.declare_dram_parameter("esrc", [128, NSUB], i32, isOutput=False)
    t_edst = nc.declare_dram_parameter("edst", [128, NSUB], f32, isOutput=False)
    t_wenc = nc.declare_dram_parameter("wenc", [128, 128], bf16, isOutput=False)
    t_benc = nc.declare_dram_parameter("benc", [128, NHID], f32, isOutput=False)
    t_wbigA = nc.declare_dram_parameter("wbigA", [128, HW], bf16, isOutput=False)
    t_wbigB = nc.declare_dram_parameter("wbigB", [128, HW], bf16, isOutput=False)
    t_vdst = nc.declare_dram_parameter("vdst", [NHID, NHEADS], f32, isOutput=False)
    t_batt = nc.declare_dram_parameter("batt", [128, 256], f32, isOutput=False)
    t_wdec = nc.declare_dram_parameter("wdec", [NHID, NCLASS], f32, isOutput=False)
    t_bdec = nc.declare_dram_parameter("bdec", [128, NCLASS], f32, isOutput=False)
    t_iota = nc.declare_dram_parameter("iota", [128, 128], f32, isOutput=False)
    t_ones4 = nc.declare_dram_parameter("ones4", [128, NHEADS], f32, isOutput=False)
    t_out = nc.declare_dram_parameter("out", [NPAD, NCLASS], f32, isOutput=True)

    h2 = nc.dram_tensor("h2tab", [NTAB, HW], bf16)
    ag_in = nc.dram_tensor("ag_in", [NHID, NPAD], bf16)
    ag_out = nc.dram_tensor("ag_out", [NCORES * NHID, NPAD], bf16,
                            addr_space="Shared")
    sub_base = np.concatenate([[0], np.cumsum(S)[:-1]]).astype(int)
    SMAX = int(max(S))

    with tile.TileContext(nc) as tc:
        with tc.tile_pool(name="const", bufs=1) as cp, \
             tc.tile_pool(name="work", bufs=4) as wp, \
             tc.tile_pool(name="gath", bufs=2) as gp, \
             tc.tile_pool(name="ohp", bufs=2) as op_, \
             tc.tile_pool(name="pacc", bufs=2, space="PSUM") as pacc, \
             tc.tile_pool(name="prot", bufs=2, space="PSUM") as prot:

            # ---- resident tiles / constants ----
            XT = cp.tile([128, 4 * NPAD], bf16)     # X^T (bf16), all cores
            XTo = cp.tile([NHID, NPAD], f32)        # own X^T (fp32 residual)
            adst = cp.tile([128, NTILES * NHEADS], f32)
            adstb = cp.tile([128, NTILES * NHEADS], bf16)
            esrc_sb = cp.tile([128, NSUB], i32)
            edst_sb = cp.tile([128, NSUB], f32)
            wenc_sb = cp.tile([128, 128], bf16)
            benc_sb = cp.tile([128, NHID], f32)
            wbigA_sb = cp.tile([128, HW], bf16)
            wbigB_sb = cp.tile([128, HW], bf16)
            vdst_sb = cp.tile([NHID, NHEADS], f32)
            batt_sb = cp.tile([128, 256], f32)
            wdec_sb = cp.tile([NHID, NCLASS], f32)
            bdec_sb = cp.tile([128, NCLASS], f32)
            iota_sb = cp.tile([128, 128], f32)
            ones4_sb = cp.tile([128, NHEADS], f32)
            identf = cp.tile([128, 128], f32)
            identb = cp.tile([128, 128], bf16)
            zero4 = cp.tile([128, NHEADS], f32)
            nc.gpsimd.memset(zero4[:], 0.0)

            nc.sync.dma_start(out=esrc_sb[:], in_=t_esrc[:, :])
            nc.sync.dma_start(out=edst_sb[:], in_=t_edst[:, :])
            nc.sync.dma_start(out=wenc_sb[:], in_=t_wenc[:, :])
            nc.sync.dma_start(out=benc_sb[:], in_=t_benc[:, :])
            nc.sync.dma_start(out=wbigA_sb[:], in_=t_wbigA[:, :])
            nc.sync.dma_start(out=wbigB_sb[:], in_=t_wbigB[:, :])
            nc.sync.dma_start(out=vdst_sb[:], in_=t_vdst[:, :])
            nc.sync.dma_start(out=batt_sb[:], in_=t_batt[:, :])
            nc.sync.dma_start(out=wdec_sb[:], in_=t_wdec[:, :])
            nc.sync.dma_start(out=bdec_sb[:], in_=t_bdec[:, :])
            nc.sync.dma_start(out=iota_sb[:], in_=t_iota[:, :])
            nc.sync.dma_start(out=ones4_sb[:], in_=t_ones4[:, :])
            make_identity(nc, identf[:])
            make_identity(nc, identb[:])

            def ts(i, n=128):
                return slice(i * n, (i + 1) * n)

            # ---- encoder: X0 = relu(x @ W_enc + b_enc) for own nodes ----
            for t in range(NTILES):
                pe = pacc.tile([128, HW], f32, tag="pp")
                for k in range(2):
                    lt = wp.tile([128, 128], bf16, tag="xload")
                    nc.sync.dma_start(out=lt[:], in_=t_xT[ts(k), ts(t)])
                    nc.tensor.matmul(pe[:, 0:NHID], lhsT=lt[:],
                                     rhs=wenc_sb[:, k * NHID:(k + 1) * NHID],
                                     start=(k == 0), stop=(k == 1))
                x0 = wp.tile([128, NHID], f32, tag="x0")
                nc.vector.tensor_add(out=x0[:], in0=pe[:, 0:NHID], in1=benc_sb[:])
                nc.scalar.activation(out=x0[:], in_=x0[:], func=AF.Relu)
                pt = prot.tile([128, 128], f32, tag="prf")
                nc.tensor.transpose(out=pt[0:NHID, 0:128], in_=x0[:],
                                    identity=identf[:])
                nc.vector.tensor_copy(out=XTo[:, ts(t)], in_=pt[0:NHID, 0:128])

            def do_allgather():
                xtb = wp.tile([NHID, NPAD], bf16, tag="xtb")
                nc.scalar.activation(out=xtb[:], in_=XTo[:], func=AF.Copy)
                nc.sync.dma_start(out=ag_in[:, :], in_=xtb[:])
                nc.gpsimd.collective_compute(
                    "AllGather", OP.bypass,
                    replica_groups=[list(range(NCORES))],
                    ins=[ag_in.ap().opt()], outs=[ag_out.ap().opt()])
                for r in range(NCORES):
                    nc.sync.dma_start(
                        out=XT[(r % 2) * NHID:(r % 2 + 1) * NHID,
                               (r // 2) * NPAD:(r // 2 + 1) * NPAD],
                        in_=ag_out[r * NHID:(r + 1) * NHID, :])

            if DBG < 2:
                do_allgather()

            # ---- layers ----
            for L in range(NL if DBG not in (4, 5, 6) else 0):
                # a_dst for own tiles (fp32, small)
                for t in range(NTILES):
                    pa = prot.tile([128, 128], f32, tag="prf")
                    nc.tensor.matmul(pa[:, 0:NHEADS], lhsT=XTo[:, ts(t)],
                                     rhs=vdst_sb[:], start=True, stop=True)
                    nc.vector.tensor_copy(out=adst[:, t * NHEADS:(t + 1) * NHEADS],
                                          in_=pa[:, 0:NHEADS])
                nc.scalar.activation(out=adstb[:], in_=adst[:], func=AF.Copy)

                # h' table sweep (all 392 global tiles), bf16
                for m in range(NTG):
                    r, lm = m // NTILES, m % NTILES
                    lhsT = XT[:][:, (r // 2) * NPAD + lm * 128:
                                 (r // 2) * NPAD + (lm + 1) * 128]
                    wb = wbigA_sb if (r % 2 == 0) else wbigB_sb
                    ph = pacc.tile([128, HW], f32, tag="pp")
                    nc.tensor.matmul(ph[:], lhsT=lhsT, rhs=wb[:],
                                     start=True, stop=True)
                    hsb = wp.tile([128, HW], bf16, tag="hsb")
                    nc.scalar.activation(out=hsb[:], in_=ph[:], func=AF.Copy)
                    # constant-1 columns per head (denominator trick)
                    nc.vector.tensor_copy(
                        out=hsb[:, 0:NHEADS * HP].rearrange(
                            "p (h c) -> p h c", c=HP)[:, :, NHID:HP],
                        in_=ones4_sb[:, :, None])
                    nc.sync.dma_start(out=h2[ts(m), :], in_=hsb[:])

                # edge sweep
                for t in range(NTILES if DBG in (0, 3) else 0):
                    St = S[t]
                    base = int(sub_base[t])
                    gt_all = gp.tile([128, SMAX * HW], bf16, tag="gt")
                    oh_all = op_.tile([128, SMAX * 128], bf16, tag="oh")
                    # one PSUM bank: [0:260] aggregation, [HW:] a_dst slots
                    po = pacc.tile([128, HW + SMAX * NHEADS], f32, tag="po")
                    pad_ps = po[:, HW:HW + SMAX * NHEADS]
                    for j in range(St):
                        kk = base + j
                        nc.gpsimd.indirect_dma_start(
                            out=gt_all[:, j * HW:(j + 1) * HW],
                            out_offset=None, in_=h2[:, :],
                            in_offset=bass.IndirectOffsetOnAxis(
                                ap=esrc_sb[:, kk:kk + 1], axis=0))
                        nc.vector.tensor_tensor(
                            out=oh_all[:, ts(j)],
                            in0=edst_sb[:, kk:kk + 1].to_broadcast([128, 128]),
                            in1=iota_sb[:], op=OP.is_equal)
                        ptr = prot.tile([128, 128], bf16, tag="prb")
                        nc.tensor.transpose(out=ptr[:], in_=oh_all[:, ts(j)],
                                            identity=identb[:])
                        oht = wp.tile([128, 128], bf16, tag="oht")
                        nc.scalar.activation(out=oht[:], in_=ptr[:], func=AF.Copy)
                        nc.tensor.matmul(
                            pad_ps[:, j * NHEADS:(j + 1) * NHEADS], lhsT=oht[:],
                            rhs=adstb[:, t * NHEADS:(t + 1) * NHEADS],
                            start=True, stop=True)
                    # batched softmax-weight chain for the whole tile:
                    # wv = exp(leakyrelu(a_src + a_dst)) via
                    # max(exp(x), exp(0.2 x)) (exp is monotone)
                    nsl = St * NHEADS
                    gt3 = gt_all[:].rearrange("p (j c) -> p j c", c=HW)
                    wv = wp.tile([128, SMAX * NHEADS], f32, tag="wv")
                    nc.vector.tensor_add(
                        out=wv[:, 0:nsl].rearrange("p (j h) -> p j h", h=NHEADS),
                        in0=gt3[:, 0:St, NHEADS * HP:HW],
                        in1=pad_ps[:, 0:nsl].rearrange("p (j h) -> p j h",
                                                       h=NHEADS))
                    e1 = wp.tile([128, SMAX * NHEADS], f32, tag="e1")
                    nc.scalar.activation(out=e1[:, 0:nsl], in_=wv[:, 0:nsl],
                                         func=AF.Exp)
                    nc.scalar.activation(out=wv[:, 0:nsl], in_=wv[:, 0:nsl],
                                         func=AF.Exp, scale=NEG_SLOPE)
                    nc.vector.tensor_tensor(out=wv[:, 0:nsl], in0=e1[:, 0:nsl],
                                            in1=wv[:, 0:nsl], op=OP.max)
                    wvb = wp.tile([128, SMAX * NHEADS], bf16, tag="wvb")
                    nc.scalar.activation(out=wvb[:, 0:nsl], in_=wv[:, 0:nsl],
                                         func=AF.Copy)
                    for j in range(St):
                        rhs = wp.tile([128, NHEADS * HP], bf16, tag="rhs")
                        nc.vector.tensor_tensor(
                            out=rhs[:].rearrange("p (h c) -> p h c", c=HP),
                            in0=gt3[:, j, 0:NHEADS * HP].rearrange(
                                "p (h c) -> p h c", c=HP),
                            in1=wvb[:, j * NHEADS:(j + 1) * NHEADS, None]
                                .to_broadcast([128, NHEADS, HP]),
                            op=OP.mult)
                        nc.tensor.matmul(po[:, 0:NHEADS * HP], lhsT=oh_all[:, ts(j)],
                                         rhs=rhs[:], start=(j == 0),
                                         stop=(j == St - 1))
                    # epilogue: alpha-normalize, bias, relu, head-mean, update
                    po3 = po[:, 0:NHEADS * HP].rearrange("p (h c) -> p h c", c=HP)
                    # den==0 only on padded dst rows; +1 there avoids Inf/NaN
                    # leaking into the pad-row residual (and from there into
                    # the a_dst matmul contraction of later layers)
                    den = wp.tile([128, NHEADS], f32, tag="den")
                    nc.vector.tensor_tensor(out=den[:, :, None],
                                            in0=po3[:, :, NHID:HP],
                                            in1=zero4[:, :, None], op=OP.is_equal)
                    nc.vector.tensor_add(out=den[:, :, None], in0=den[:, :, None],
                                         in1=po3[:, :, NHID:HP])
                    rcp = wp.tile([128, NHEADS], f32, tag="rcp")
                    nc.vector.reciprocal(out=rcp[:], in_=den[:])
                    conv = wp.tile([128, 256], f32, tag="conv")
                    nc.vector.tensor_tensor(
                        out=conv[:].rearrange("p (h c) -> p h c", c=NHID),
                        in0=po3[:, :, 0:NHID],
                        in1=rcp[:, :, None].to_broadcast([128, NHEADS, NHID]),
                        op=OP.mult)
                    nc.vector.tensor_add(out=conv[:], in0=conv[:], in1=batt_sb[:])
                    nc.scalar.activation(out=conv[:], in_=conv[:], func=AF.Relu)
                    upd = wp.tile([128, NHID], f32, tag="upd")
                    nc.vector.tensor_reduce(
                        out=upd[:], in_=conv[:].rearrange("p (j u) -> p j u", u=4),
                        axis=AX.X, op=OP.add)
                    nc.scalar.activation(out=upd[:], in_=upd[:], func=AF.Copy,
                                         scale=0.25)
                    ptu = prot.tile([128, 128], f32, tag="prf")
                    nc.tensor.transpose(out=ptu[0:NHID, 0:128], in_=upd[:],
                                        identity=identf[:])
                    nc.vector.tensor_add(out=XTo[:, ts(t)], in0=XTo[:, ts(t)],
                                         in1=ptu[0:NHID, 0:128])
                if L < NL - 1 and DBG < 2:
                    do_allgather()

            # ---- decoder ----
            for t in range(NTILES):
                pd = pacc.tile([128, HW], f32, tag="pp")
                nc.tensor.matmul(pd[:, 0:NCLASS], lhsT=XTo[:, ts(t)],
                                 rhs=wdec_sb[:], start=True, stop=True)
                dsb = wp.tile([128, NCLASS], f32, tag="dsb")
                nc.vector.tensor_add(out=dsb[:], in0=pd[:, 0:NCLASS], in1=bdec_sb[:])
                nc.sync.dma_start(out=t_out[ts(t), :], in_=dsb[:])

    nc.compile()
    return nc


def kernel(**inputs) -> np.ndarray:
    from concourse.bass_utils import run_bass_kernel_spmd

    in_maps, S, NSUB = _prepare(inputs)
    nc = _build(S, NSUB)
    trace = os.environ.get("GAT_TRACE", "0") == "1"
    ncores_run = int(os.environ.get("GAT_CORES", NCORES))
    r = run_bass_kernel_spmd(nc, in_maps[:ncores_run],
                             core_ids=list(range(ncores_run)), trace=trace)
    if trace and r.exec_time_ns is not None:
        print(f"HW exec time: {r.exec_time_ns} ns")
    out = np.concatenate(
        [r.results[c]["out"][:NPER] for c in range(len(r.results))], axis=0)
    if len(r.results) < NCORES:
        out = np.pad(out, ((0, N - out.shape[0]), (0, 0)))
    return out.astype(np.float32)


if __name__ == "__main__":
    import reference as R
    inputs = R.setup_inputs()
    out = kernel(**{k: np.asarray(v) for k, v in inputs.items()})
    print("out shape:", out.shape)
